# revision 44
# baseline (speedup 1.0000x reference)
"""Distributed 2-layer GCN (BangaloreGCN) on 8 Trainium2 NeuronCores.

v3 strategy (node/graph-parallel):
  * Nodes packed into 8*49 destination bins of 128 lanes (LPT on
    in-degree).  GCN refactored so message passing is gather + one-hot
    matmul segment-sum: out = dinv*(A @ (dinv*h)) + dinv^2*h, BN scale
    folded into W, biases applied channel-major after a PE transpose.
  * Layer tables are densely packed (2 or 4 node rows per 256B gather
    line) and replicated with AllGather collectives split into two
    source-halves (tile slots [0,TA) / [TA,49)) so the second collective
    overlaps compute:
      - L1: AG1A starts right after the first TA dense tiles; the L1
        scatter is two passes (A-half sources while AG1B transfers,
        then B-half sources + PSUM-spill re-add).
      - L2: W2 applied before the collective (32-wide table); AG2A is
        issued mid-way through the L1 B-pass; the L2 scatter is likewise
        two passes so the A-half scatter hides under AG2B.
  * dma_gather streams per source-class (slot parity) with per-tile
    compile-time chunk budgets (max over cores); greedy host-side
    class balancing.  Sel one-hot masks are built chunk-innermost so
    the DVE 2x perf mode applies.
"""

import sys

sys.path.insert(0, "/opt/trn_rl_repo")

import heapq

import ml_dtypes
import numpy as np

BF16 = ml_dtypes.bfloat16

# ---- problem constants (hardcoded per contest contract) ----
N_NODES = 50000
IN_CH = 128
HID = 64
HID2 = 32
BN_EPS = 1e-5

NCORES = 8
P = 128
TILES = 49                 # dest tiles per core
SPC = TILES * P            # slots per core (6272)
NSLOT = NCORES * SPC       # 50176
NBINS = NCORES * TILES     # 392
TA = 26                    # tiles in half A
TB = TILES - TA            # 25
SZA = TA * P               # 3072 slots per core in half A
SZB = TB * P               # 3200
NLA1 = NCORES * SZA // 2   # 12288 L1 lines in table A
NLB1 = NCORES * SZB // 2   # 12800
NL2A = NCORES * SZA // 4   # L2 lines in table A
NL2B = NCORES * SZB // 4
PAD_DEST = 200.0


# ----------------------------------------------------------------------
# host-side preparation
# ----------------------------------------------------------------------
def _pack_nodes_bins(deg_in, n):
    """LPT-pack nodes into NBINS bins of <=128 by in-degree."""
    order = np.argsort(-deg_in, kind="stable")
    heap = [(0, b) for b in range(NBINS)]
    heapq.heapify(heap)
    counts = np.zeros(NBINS, np.int32)
    binof = np.empty(n, np.int32)
    for v in order:
        load, b = heapq.heappop(heap)
        binof[v] = b
        counts[b] += 1
        if counts[b] < P:
            heapq.heappush(heap, (load + int(deg_in[v]), b))
    return binof


def _assign_classes(row, col_bin, out_deg, binof, h_node, n):
    """Greedy mod-4 class per node, balancing per-(dest bin, class,
    source-half) edge counts subject to 32 lanes per class per bin."""
    order_e = np.argsort(row, kind="stable")
    s_sorted = row[order_e]
    starts = np.searchsorted(s_sorted, np.arange(n))
    ends = np.searchsorted(s_sorted, np.arange(n) + 1)
    dbins_sorted = col_bin[order_e]
    cnt = np.zeros((NBINS, 4, 2), np.int64)
    cap = np.full((NBINS, 4), P // 4, np.int32)
    cls = np.empty(n, np.int8)
    for v in np.argsort(-out_deg, kind="stable"):
        b = binof[v]
        h = h_node[v]
        db = dbins_sorted[starts[v]:ends[v]]
        if len(db):
            sc = cnt[db, :, h].sum(axis=0)
        else:
            sc = np.zeros(4, np.int64)
        sc = np.where(cap[b] > 0, sc, 1 << 40)
        c = int(np.argmin(sc))
        cls[v] = c
        cap[b, c] -= 1
        if len(db):
            np.add.at(cnt, (db, c, h), 1)
    return cls


def _wrap_idx(arr):
    ni = arr.shape[0]
    blk = arr.reshape(ni // 16, 16).T.astype(np.int16)
    return np.tile(blk, (8, 1))


def host_prep(x, edge_index, W1, b1, W2, b2, fcW, fcb,
              g1, be1, rm1, rv1, g2, be2, rm2, rv2):
    n = x.shape[0]
    row = np.asarray(edge_index[0], np.int64)
    col = np.asarray(edge_index[1], np.int64)

    deg = np.bincount(col, minlength=n).astype(np.float32) + 1.0
    dinv = (1.0 / np.sqrt(deg)).astype(np.float32)
    deg_in = np.bincount(col, minlength=n)
    deg_out = np.bincount(row, minlength=n)

    binof = _pack_nodes_bins(deg_in, n)

    # per-core: sort own bins desc by in-degree -> tile slots
    bin_in = np.bincount(binof[col], minlength=NBINS)
    tslot_of_bin = np.empty(NBINS, np.int64)
    for c in range(NCORES):
        bins = np.arange(c * TILES, (c + 1) * TILES)
        order_b = bins[np.argsort(-bin_in[bins], kind="stable")]
        tslot_of_bin[order_b] = np.arange(TILES)
    rbin = (np.arange(NBINS) // TILES) * TILES + tslot_of_bin

    h_node = (tslot_of_bin[binof] >= TA).astype(np.int8)   # source half per node
    cls4 = _assign_classes(row, rbin[binof[col]], deg_out, binof, h_node, n)

    # lanes: class c gets lanes {c, c+4, ...} within its bin
    lane = np.empty(n, np.int64)
    key = binof.astype(np.int64) * 4 + cls4
    order = np.argsort(key, kind="stable")
    uniq, first = np.unique(key[order], return_index=True)
    rank = np.arange(n) - np.repeat(first, np.diff(np.append(first, n)))
    lane[order] = cls4[order] + 4 * rank
    assert lane.max() < P

    slot_of_node = rbin[binof] * P + lane          # table slot (core-major)
    node_of_slot = np.full(NSLOT, -1, np.int64)
    node_of_slot[slot_of_node] = np.arange(n)

    src_slot = slot_of_node[row]
    dst_slot = slot_of_node[col]
    dlane = dst_slot % P
    scls = (src_slot % 4).astype(np.int64)

    core_of_edge = (dst_slot // P) // TILES
    tslot = (dst_slot // P) % TILES

    # source half + half-local slot / lines
    s_core = src_slot // SPC
    s_t = (src_slot % SPC) // P
    s_lane = src_slot % P
    s_h = (s_t >= TA).astype(np.int64)
    sH = np.where(s_h == 0,
                  s_core * SZA + s_t * P + s_lane,
                  s_core * SZB + (s_t - TA) * P + s_lane)
    line1 = sH >> 1                                # within half table
    line2 = sH >> 2                                # within half table

    # budgets
    cnt1 = np.zeros((NCORES, TILES, 2, 2), np.int64)   # [core, t, cls2, half]
    np.add.at(cnt1, (core_of_edge, tslot, scls % 2, s_h), 1)
    c1h = -(-cnt1.max(axis=0) // P)                    # [TILES, 2, 2]
    cnt4 = np.zeros((NCORES, TILES, 4, 2), np.int64)
    np.add.at(cnt4, (core_of_edge, tslot, scls, s_h), 1)
    c2h = -(-cnt4.max(axis=0) // P)                    # [TILES, 4, 2]

    # sort edges by (core, tslot, cls4, half)
    ekey = (((core_of_edge * TILES + tslot) * 4 + scls) * 2 + s_h)
    eorder = np.argsort(ekey, kind="stable")
    e_line1 = line1[eorder]
    e_line2 = line2[eorder]
    e_dlane = dlane[eorder]
    e_key = ekey[eorder]
    bounds = np.searchsorted(e_key, np.arange(NCORES * TILES * 8 + 1))

    S1c = (g1 / np.sqrt(rv1 + BN_EPS)).astype(np.float32)
    T1 = ((b1 - rm1) * S1c + be1).astype(np.float32)
    S2c = (g2 / np.sqrt(rv2 + BN_EPS)).astype(np.float32)
    T2 = ((b2 - rm2) * S2c + be2).astype(np.float32)
    W1p = (W1 * S1c[None, :]).astype(np.float32)
    W2p = (W2 * S2c[None, :]).astype(np.float32)

    NCH1 = [int(c1h[:, :, h].sum()) for h in range(2)]   # dest-img cols per half
    NCH2 = [int(c2h[:, :, h].sum()) for h in range(2)]

    cores = []
    for c in range(NCORES):
        idx1 = [[np.zeros(int(c1h[:, a, h].sum()) * P, np.int64)
                 for h in range(2)] for a in range(2)]
        idx2 = [[np.zeros(int(c2h[:, k, h].sum()) * P, np.int64)
                 for h in range(2)] for k in range(4)]
        dest1 = [np.full((NCH1[h], P), PAD_DEST, np.float32) for h in range(2)]
        dest2 = [np.full((NCH2[h], P), PAD_DEST, np.float32) for h in range(2)]
        off1 = [[0, 0], [0, 0]]
        off2 = [[0, 0], [0, 0], [0, 0], [0, 0]]
        col1 = [0, 0]
        col2 = [0, 0]

        def sl(t, k, h):
            i = ((c * TILES + t) * 4 + k) * 2 + h
            return bounds[i], bounds[i + 1]

        for t in range(TILES):
            for h in range(2):
                for k in range(4):
                    lo, hi = sl(t, k, h)
                    li = e_line2[lo:hi]
                    dl = e_dlane[lo:hi]
                    cap = int(c2h[t, k, h]) * P
                    assert len(li) <= cap
                    idx2[k][h][off2[k][h]:off2[k][h] + len(li)] = li
                    d = dest2[h][col2[h]:col2[h] + c2h[t, k, h]].reshape(-1)
                    d[:len(li)] = dl
                    off2[k][h] += cap
                    col2[h] += int(c2h[t, k, h])
            for h in range(2):
                for a in range(2):
                    parts = []
                    for k in (a, a + 2):
                        lo, hi = sl(t, k, h)
                        parts.append((e_line1[lo:hi], e_dlane[lo:hi]))
                    li = np.concatenate([p[0] for p in parts])
                    dl = np.concatenate([p[1] for p in parts])
                    cap = int(c1h[t, a, h]) * P
                    assert len(li) <= cap
                    idx1[a][h][off1[a][h]:off1[a][h] + len(li)] = li
                    d = dest1[h][col1[h]:col1[h] + c1h[t, a, h]].reshape(-1)
                    d[:len(li)] = dl
                    off1[a][h] += cap
                    col1[h] += int(c1h[t, a, h])

        nodes = node_of_slot[c * SPC:(c + 1) * SPC]
        occ = nodes >= 0
        xs = np.zeros((SPC, IN_CH), np.float32)
        xs[occ] = x[nodes[occ]]
        dv = np.zeros(SPC, np.float32)
        dv[occ] = dinv[nodes[occ]]

        cores.append(dict(
            idx1=[[_wrap_idx(idx1[a][h]) for h in range(2)] for a in range(2)],
            idx2=[[_wrap_idx(idx2[k][h]) for h in range(2)] for k in range(4)],
            dest1=[dest1[h].T.astype(BF16).copy() for h in range(2)],
            dest2=[dest2[h].T.astype(BF16).copy() for h in range(2)],
            xT=np.ascontiguousarray(xs.T),
            dinv=np.ascontiguousarray(dv.reshape(TILES, P).T),
            nodes=nodes,
        ))

    consts = dict(W1p=W1p, W2p=W2p, T1=T1, T2=T2,
                  fcW=np.asarray(fcW, np.float32),
                  fcb=float(np.asarray(fcb).reshape(-1)[0]),
                  c1h=c1h, c2h=c2h)
    return cores, consts


# ----------------------------------------------------------------------
# device program
# ----------------------------------------------------------------------
def _dma_gather_raw(gp, bassmod, out_ap, in_ap, idxs_ap, num_idxs, elem_size,
                    elem_step, single_packet=True, queue_num=0):
    """bass.dma_gather with elem_size_bytes below 256B allowed (stride must
    still be a multiple of 256B)."""
    import concourse.mybir as mybir
    from concourse import ap_utils
    from concourse.bass import MemorySpace, exact_div, round_up_to_multiple

    assert idxs_ap.dtype == mybir.dt.int16
    assert in_ap.dtype == out_ap.dtype
    assert in_ap.space == MemorySpace.DRAM
    assert idxs_ap.space == MemorySpace.SBUF and out_ap.space == MemorySpace.SBUF
    assert ap_utils.ap_is_contiguous(out_ap.ap[1:])
    assert ap_utils.ap_is_contiguous(idxs_ap.ap[1:])
    assert in_ap.ap[-1][1] == out_ap.ap[-1][1] == elem_size
    assert out_ap.ap[0][1] * out_ap.ap[1][1] == round_up_to_multiple(num_idxs, 128)
    assert in_ap.ap[0][0] == elem_step
    stride_bytes_256 = exact_div(elem_step * mybir.dt.size(in_ap.dtype), 256)
    assert stride_bytes_256 < 256
    return gp.add_instruction(
        mybir.InstDMAGatherAnt(
            name=bassmod.get_next_instruction_name(),
            ins=[*gp.lower_ap_dma(in_ap, for_custom_bir_dma=True),
                 gp.lower_ap(idxs_ap),
                 gp.lower_val_access(gp.to_reg(num_idxs))],
            outs=[gp.lower_ap(out_ap)],
            transpose=False,
            num_idxs=num_idxs,
            elem_size=elem_size,
            stride_bytes_256=stride_bytes_256,
            gen_mode=0,
            single_packet=single_packet,
            queue_num=queue_num,
            sbuf_tokens_per_rank=0,
            sbuf_free_dim_per_rank=0,
            sbuf_free_dim_pad_per_rank=0,
            sbuf_byte_offset=0,
        ))


GRP1 = [10, 10, 10, 10, 9]     # gather groups (tiles per group)
GRP1B = GRP1
GRP2 = [13, 12, 12, 12]        # L2 pass groups


def build_bass(c1h, c2h):
    import concourse.bacc as bacc
    import concourse.bass as bassm
    import concourse.mybir as mybir
    import concourse.tile as tile
    from concourse.library_config import mlp
    from concourse.masks import make_identity

    f32 = mybir.dt.float32
    bf = mybir.dt.bfloat16
    i16 = mybir.dt.int16
    AF = mybir.ActivationFunctionType

    c1h = [[[int(c1h[t][a][h]) for h in range(2)] for a in range(2)]
           for t in range(TILES)]
    c2h = [[[int(c2h[t][k][h]) for h in range(2)] for k in range(4)]
           for t in range(TILES)]
    nch1 = [[sum(c1h[t][a][h] for a in range(2)) for h in range(2)]
            for t in range(TILES)]
    nch2 = [[sum(c2h[t][k][h] for k in range(4)) for h in range(2)]
            for t in range(TILES)]
    NCH1 = [sum(nch1[t][h] for t in range(TILES)) for h in range(2)]
    NCH2 = [sum(nch2[t][h] for t in range(TILES)) for h in range(2)]
    NCHM = max(max(max(nch1[t]) for t in range(TILES)),
               max(max(nch2[t]) for t in range(TILES)))
    w1tot = [[sum(c1h[t][a][h] for t in range(TILES)) * 8 for h in range(2)]
             for a in range(2)]
    w2tot = [[sum(c2h[t][k][h] for t in range(TILES)) * 8 for h in range(2)]
             for k in range(4)]

    nc = bacc.Bacc("TRN2", target_bir_lowering=False)
    xT_d = nc.dram_tensor("xT", [P, SPC], bf, kind="ExternalInput")
    idx1_d = [[nc.dram_tensor(f"idx1_{a}_{h}", [P, w1tot[a][h]], i16,
                              kind="ExternalInput") for h in range(2)]
              for a in range(2)]
    idx2_d = [[nc.dram_tensor(f"idx2_{k}_{h}", [P, w2tot[k][h]], i16,
                              kind="ExternalInput") for h in range(2)]
              for k in range(4)]
    dest1_d = [nc.dram_tensor(f"dest1_{h}", [P, NCH1[h]], bf,
                              kind="ExternalInput") for h in range(2)]
    dest2_d = [nc.dram_tensor(f"dest2_{h}", [P, NCH2[h]], bf,
                              kind="ExternalInput") for h in range(2)]
    dinv_d = nc.dram_tensor("dinv", [P, TILES], f32, kind="ExternalInput")
    w1_d = nc.dram_tensor("w1", [IN_CH, HID], bf, kind="ExternalInput")
    w2_d = nc.dram_tensor("w2", [HID, HID2], f32, kind="ExternalInput")
    t1_d = nc.dram_tensor("t1", [HID, 1], f32, kind="ExternalInput")
    t2_d = nc.dram_tensor("t2", [HID2, 1], f32, kind="ExternalInput")
    fcw_d = nc.dram_tensor("fcw", [HID2, 1], f32, kind="ExternalInput")
    y_d = nc.dram_tensor("y", [P, TILES], f32, kind="ExternalOutput")

    with tile.TileContext(nc) as tc:
        with (
            tc.tile_pool(name="const", bufs=1) as cpool,
            tc.tile_pool(name="upart", bufs=1) as upool,
            tc.tile_pool(name="g1", bufs=3) as g1pool,
            tc.tile_pool(name="g2", bufs=3) as g2pool,
            tc.tile_pool(name="sel", bufs=10) as selpool,
            tc.tile_pool(name="work", bufs=6) as wpool,
            tc.tile_pool(name="pacc", bufs=3, space="PSUM") as pacc,
            tc.tile_pool(name="pmm", bufs=2, space="PSUM") as pmm,
            tc.tile_pool(name="ptr", bufs=2, space="PSUM") as ptr,
            tc.tile_pool(name="dram", bufs=1, space="DRAM") as dpool,
        ):
            nc.gpsimd.load_library(mlp)

            # ---- tensors needed for the dense stage first ----
            dinv_t = cpool.tile([P, TILES], f32)
            nc.sync.dma_start(out=dinv_t[:], in_=dinv_d[:])
            w1_t = cpool.tile([IN_CH, HID], bf)
            nc.sync.dma_start(out=w1_t[:], in_=w1_d[:])
            xfull = cpool.tile([P, SPC], bf)
            nc.sync.dma_start(out=xfull[:, :SZA], in_=xT_d[:, :SZA])
            nc.sync.dma_start(out=xfull[:, SZA:], in_=xT_d[:, SZA:])

            tab1_t = upool.tile([P, TILES, HID], bf, tag="tab1")
            tab2_t = upool.tile([P, TILES, HID2], bf, tag="tab2")
            zsp_t = upool.tile([P, TILES, HID], f32, tag="zsp")
            zsp2_t = upool.tile([P, TILES, HID2], f32, tag="zsp2")
            out_t = upool.tile([P, TILES], f32, tag="out")

            ag1A = dpool.tile([SZA, HID], bf, name="ag1A")
            ag1B = dpool.tile([SZB, HID], bf, name="ag1B")
            s1A = dpool.tile([NLA1, 2 * HID], bf, addr_space="Shared",
                             name="s1A")
            s1B = dpool.tile([NLB1, 2 * HID], bf, addr_space="Shared",
                             name="s1B")
            ag2A = dpool.tile([SZA, HID2], bf, name="ag2A")
            ag2B = dpool.tile([SZB, HID2], bf, name="ag2B")
            s2A = dpool.tile([NL2A, 4 * HID2], bf, addr_space="Shared",
                             name="s2A")
            s2B = dpool.tile([NL2B, 4 * HID2], bf, addr_space="Shared",
                             name="s2B")

            # ---- L1 dense: tab1 = dinv * (x @ W1'), half A then B ----
            def dense(t0, t1r):
                for t in range(t0, t1r):
                    pm = pmm.tile([P, HID], f32, space="PSUM", tag="pm")
                    nc.tensor.matmul(out=pm[:], lhsT=xfull[:, t * P:(t + 1) * P],
                                     rhs=w1_t[:], start=True, stop=True)
                    nc.scalar.activation(out=tab1_t[:, t, :], in_=pm[:],
                                         func=AF.Copy, scale=dinv_t[:, t:t + 1])

            dense(0, TA)
            nc.sync.dma_start(
                out=ag1A[:].rearrange("(t p) w -> p t w", p=P),
                in_=tab1_t[:, :TA, :])
            nc.gpsimd.collective_compute(
                "AllGather", mybir.AluOpType.bypass,
                replica_groups=[list(range(NCORES))],
                ins=[ag1A[:]], outs=[s1A[:]])
            dense(TA, TILES)
            nc.sync.dma_start(
                out=ag1B[:].rearrange("(t p) w -> p t w", p=P),
                in_=tab1_t[:, TA:, :])
            nc.gpsimd.collective_compute(
                "AllGather", mybir.AluOpType.bypass,
                replica_groups=[list(range(NCORES))],
                ins=[ag1B[:]], outs=[s1B[:]])

            # ---- remaining constants (overlap with AG1A/AG1B) ----
            idx1_t = [[cpool.tile([P, w1tot[a][h]], i16, name=f"idx1t{a}{h}")
                       for h in range(2)] for a in range(2)]
            idx2_t = [[cpool.tile([P, w2tot[k][h]], i16, name=f"idx2t{k}{h}")
                       for h in range(2)] for k in range(4)]
            for a in range(2):
                for h in range(2):
                    nc.sync.dma_start(out=idx1_t[a][h][:], in_=idx1_d[a][h][:])
            for k in range(4):
                for h in range(2):
                    nc.sync.dma_start(out=idx2_t[k][h][:], in_=idx2_d[k][h][:])
            dest1_t = [cpool.tile([P, NCH1[h]], bf, name=f"dest1t{h}")
                       for h in range(2)]
            for h in range(2):
                nc.sync.dma_start(out=dest1_t[h][:], in_=dest1_d[h][:])
            dest2_t = [cpool.tile([P, NCH2[h]], bf, name=f"dest2t{h}")
                       for h in range(2)]
            for h in range(2):
                nc.sync.dma_start(out=dest2_t[h][:], in_=dest2_d[h][:])
            w2_t = cpool.tile([HID, HID2], f32)
            nc.sync.dma_start(out=w2_t[:], in_=w2_d[:])
            t1_t = cpool.tile([HID, 1], f32)
            nc.sync.dma_start(out=t1_t[:], in_=t1_d[:])
            t2_t = cpool.tile([HID2, 1], f32)
            nc.sync.dma_start(out=t2_t[:], in_=t2_d[:])
            fcw_t = cpool.tile([HID2, 1], f32)
            nc.sync.dma_start(out=fcw_t[:], in_=fcw_d[:])

            identf = cpool.tile([P, P], f32)
            make_identity(nc, identf[:])
            identb = cpool.tile([P, P], bf)
            nc.vector.tensor_copy(out=identb[:], in_=identf[:])
            # iota2[p, j, c] = j  (lane on middle axis, chunk innermost)
            iota_i = cpool.tile([P, P * NCHM], mybir.dt.int32)
            nc.gpsimd.iota(iota_i[:], pattern=[[1, P], [0, NCHM]], base=0,
                           channel_multiplier=0)
            iota_b = cpool.tile([P, P * NCHM], bf)
            nc.vector.tensor_copy(out=iota_b[:], in_=iota_i[:])
            iota_r = iota_b[:].rearrange("p (j c) -> p j c", c=NCHM)

            def tab_ap(tab, nlines, sub_off, elem):
                return bassm.AP(tensor=tab[:].tensor, offset=sub_off,
                                ap=[[2 * HID, nlines], [1, elem]])

            def sel_build(dest_t, dcol, nch_t):
                sel = selpool.tile([P, P, NCHM], bf, tag="sel")
                nc.vector.tensor_tensor(
                    out=sel[:, :, :nch_t],
                    in0=dest_t[:, None, dcol:dcol + nch_t]
                        .to_broadcast([P, P, nch_t]),
                    in1=iota_r[:, :, :nch_t],
                    op=mybir.AluOpType.is_equal)
                return sel

            # ---- generic scatter pass ----
            def scatter(groups, tab_of_cls, nlines_of_cls, elem, ncls, cbud,
                        idx_t, dest_t, ga_pool, ga_tag, finish, checkpoints=()):
                goff = [0] * ncls
                dcol = 0
                gmax = max(sum(cbud[t][k] for t in range(g0, g0 + gn))
                           for g0, gn in _spans(groups)
                           for k in range(ncls))
                t0 = 0
                for gn in groups:
                    gas = []
                    for k in range(ncls):
                        gw = sum(cbud[t][k] for t in range(t0, t0 + gn))
                        ga = ga_pool.tile([P, gmax, elem], bf,
                                          tag=f"{ga_tag}_{k}")
                        ni = gw * P
                        if ni:
                            _dma_gather_raw(
                                nc.gpsimd, nc, ga[:, :gw, :],
                                tab_ap(tab_of_cls(k), nlines_of_cls(k),
                                       (k % 2) * elem if elem == HID else k * elem,
                                       elem),
                                idx_t[k][:, goff[k]:goff[k] + ni // 16],
                                ni, elem, 2 * HID, single_packet=False)
                        goff[k] += ni // 16
                        gas.append(ga)
                    coff = [0] * ncls
                    for t in range(t0, t0 + gn):
                        nch_t = sum(cbud[t])
                        acc = pacc.tile([P, elem], f32, space="PSUM", tag="acc")
                        if nch_t:
                            sel = sel_build(dest_t, dcol, nch_t)
                        cc = 0
                        for k in range(ncls):
                            for i in range(cbud[t][k]):
                                nc.tensor.matmul(
                                    out=acc[:], lhsT=sel[:, :, cc],
                                    rhs=gas[k][:, coff[k] + i, :],
                                    start=(cc == 0), stop=False)
                                cc += 1
                            coff[k] += cbud[t][k]
                        dcol += nch_t
                        finish(t, acc, cc == 0)
                        if t + 1 in checkpoints:
                            checkpoints[t + 1]()
                    t0 += gn

            def _spans(groups):
                t0 = 0
                for gn in groups:
                    yield t0, gn
                    t0 += gn

            # ---- L1 pass A: accumulate A-half sources + self, spill ----
            def finA(t, acc, empty):
                nc.tensor.matmul(out=acc[:], lhsT=identb[:],
                                 rhs=tab1_t[:, t, :], start=empty, stop=True)
                nc.scalar.activation(out=zsp_t[:, t, :], in_=acc[:],
                                     func=AF.Copy)

            scatter(GRP1, lambda k: s1A, lambda k: NLA1, HID, 2,
                    [[c1h[t][0][0], c1h[t][1][0]] for t in range(TILES)],
                    [idx1_t[0][0], idx1_t[1][0]], dest1_t[0][:], g1pool, "ga1",
                    finA)

            # ---- L1 pass B: B-half sources + spill re-add, post ----
            def post1(t, acc, empty):
                nc.tensor.matmul(out=acc[:], lhsT=identf[:],
                                 rhs=zsp_t[:, t, :], start=empty, stop=True)
                z = wpool.tile([P, HID], f32, tag="z1")
                nc.scalar.activation(out=z[:], in_=acc[:], func=AF.Copy,
                                     scale=dinv_t[:, t:t + 1])
                zt = ptr.tile([HID, P], f32, space="PSUM", tag="zt")
                nc.tensor.transpose(out=zt[:], in_=z[:], identity=identf[:])
                h2 = wpool.tile([HID, P], f32, tag="h2T")
                nc.scalar.activation(out=h2[:], in_=zt[:], func=AF.Relu,
                                     bias=t1_t[:])
                p2 = pmm.tile([P, HID2], f32, space="PSUM", tag="pm")
                nc.tensor.matmul(out=p2[:], lhsT=h2[:], rhs=w2_t[:],
                                 start=True, stop=True)
                nc.scalar.activation(out=tab2_t[:, t, :], in_=p2[:],
                                     func=AF.Copy, scale=dinv_t[:, t:t + 1])

            def issue_ag2A():
                nc.sync.dma_start(
                    out=ag2A[:].rearrange("(t p) w -> p t w", p=P),
                    in_=tab2_t[:, :TA, :])
                nc.gpsimd.collective_compute(
                    "AllGather", mybir.AluOpType.bypass,
                    replica_groups=[list(range(NCORES))],
                    ins=[ag2A[:]], outs=[s2A[:]])

            scatter(GRP1B, lambda k: s1B, lambda k: NLB1, HID, 2,
                    [[c1h[t][0][1], c1h[t][1][1]] for t in range(TILES)],
                    [idx1_t[0][1], idx1_t[1][1]], dest1_t[1][:], g1pool, "ga1",
                    post1, checkpoints={TA: issue_ag2A})

            nc.sync.dma_start(
                out=ag2B[:].rearrange("(t p) w -> p t w", p=P),
                in_=tab2_t[:, TA:, :])
            nc.gpsimd.collective_compute(
                "AllGather", mybir.AluOpType.bypass,
                replica_groups=[list(range(NCORES))],
                ins=[ag2B[:]], outs=[s2B[:]])

            # ---- L2 pass A: A-half sources + self, spill ----
            def finA2(t, acc, empty):
                nc.tensor.matmul(out=acc[:], lhsT=identb[:],
                                 rhs=tab2_t[:, t, :], start=empty, stop=True)
                nc.scalar.activation(out=zsp2_t[:, t, :], in_=acc[:],
                                     func=AF.Copy)

            scatter(GRP2, lambda k: s2A, lambda k: NL2A, HID2, 4,
                    [[c2h[t][k][0] for k in range(4)] for t in range(TILES)],
                    [idx2_t[k][0] for k in range(4)], dest2_t[0][:], g2pool,
                    "ga2", finA2)

            # ---- L2 pass B: B-half sources + spill re-add, post ----
            def post2(t, acc, empty):
                nc.tensor.matmul(out=acc[:], lhsT=identf[:],
                                 rhs=zsp2_t[:, t, :], start=empty, stop=True)
                z = wpool.tile([P, HID2], f32, tag="z2")
                nc.scalar.activation(out=z[:], in_=acc[:], func=AF.Copy,
                                     scale=dinv_t[:, t:t + 1])
                zt = ptr.tile([HID2, P], f32, space="PSUM", tag="zt")
                nc.tensor.transpose(out=zt[:], in_=z[:], identity=identf[:])
                h3 = wpool.tile([HID2, P], f32, tag="h3T")
                nc.scalar.activation(out=h3[:], in_=zt[:], func=AF.Relu,
                                     bias=t2_t[:])
                py = pmm.tile([P, 1], f32, space="PSUM", tag="pm")
                nc.tensor.matmul(out=py[:], lhsT=h3[:], rhs=fcw_t[:],
                                 start=True, stop=True)
                nc.scalar.activation(out=out_t[:, t:t + 1], in_=py[:],
                                     func=AF.Copy)

            scatter(GRP2, lambda k: s2B, lambda k: NL2B, HID2, 4,
                    [[c2h[t][k][1] for k in range(4)] for t in range(TILES)],
                    [idx2_t[k][1] for k in range(4)], dest2_t[1][:], g2pool,
                    "ga2", post2)

            nc.sync.dma_start(out=y_d[:], in_=out_t[:])

    nc.compile()
    return nc


# ----------------------------------------------------------------------
# entry point
# ----------------------------------------------------------------------
def prepare(inputs):
    inputs = {k: np.asarray(v) for k, v in inputs.items()}
    cores, consts = host_prep(**inputs)
    nc = build_bass(consts["c1h"], consts["c2h"])

    t1 = consts["T1"].reshape(HID, 1).astype(np.float32)
    t2 = consts["T2"].reshape(HID2, 1).astype(np.float32)
    fcw = consts["fcW"].reshape(HID2, 1).astype(np.float32)

    in_maps = []
    for c in range(NCORES):
        m = {
            "xT": cores[c]["xT"].astype(BF16),
            "dinv": cores[c]["dinv"],
            "w1": consts["W1p"].astype(BF16),
            "w2": consts["W2p"].astype(np.float32),
            "t1": t1,
            "t2": t2,
            "fcw": fcw,
        }
        for h in range(2):
            m[f"dest1_{h}"] = cores[c]["dest1"][h]
            m[f"dest2_{h}"] = cores[c]["dest2"][h]
            for a in range(2):
                m[f"idx1_{a}_{h}"] = cores[c]["idx1"][a][h]
            for k in range(4):
                m[f"idx2_{k}_{h}"] = cores[c]["idx2"][k][h]
        in_maps.append(m)
    return nc, in_maps, consts | {"cores": cores}


def execute(nc, in_maps):
    from concourse.bass_utils import run_bass_kernel_spmd
    return run_bass_kernel_spmd(nc, in_maps, core_ids=list(range(NCORES)))


def unshard(res, consts):
    y = np.zeros((N_NODES, 1), np.float32)
    fcb = consts["fcb"]
    for c in range(NCORES):
        nodes = consts["cores"][c]["nodes"]
        occ = nodes >= 0
        vals = res.results[c]["y"].T.reshape(-1) + fcb
        y[nodes[occ], 0] = vals[occ]
    return y


def kernel(**inputs):
    nc, in_maps, consts = prepare(inputs)
    res = execute(nc, in_maps)
    return unshard(res, consts)
